# revision 30
# baseline (speedup 1.0000x reference)
"""Trainium2 Bass kernel for a 4-layer dense transformer (B=2, S=1024, D=1024, H=16).

Sharding: context-parallel over tokens across 8 cores (256 tokens/core;
cores 0-3 = batch 0, cores 4-7 = batch 1). Per layer, K and V are exchanged
within each 4-core batch group via two AllGathers (K first, so score compute
overlaps the V exchange); everything else is local.

Layouts: feature-major residual h^T [D, T]; GEMMs consume weights in native
[in, out] layout as the stationary operand (fp16, fp32 PSUM accumulate).
Attention: Q/K are fp8e4 in a per-head [32 partition rows x 2 free slots]
layout so scores run as one DoubleRow matmul per (head, key chunk). Causal
masking: key chunks never straddle a core's query window, so remote chunks
take a per-chunk-pair constant bias folded into the softmax exp (own chunks
masked out there), and a core-independent triangular mask handles the
diagonal via a separate own-chunk pass on the pre-AllGather K/V. The softmax
denominator rides along as a 65th ones-column of V. PSUM is managed as
2-bank [128, 1024] tiles so exp / gelu / residual adds run as single wide
ops. Residual, stats fp32.
"""

import sys
import os

for _p in ("/opt/trn_rl_repo", "/root/.axon_site/_ro/trn_rl_repo"):
    if os.path.isdir(_p) and _p not in sys.path:
        sys.path.insert(0, _p)

import numpy as np
import ml_dtypes as _mld
import concourse.bass as bass
import concourse.bacc as bacc
import concourse.mybir as mybir
import concourse.tile as tile
from concourse.bass_utils import run_bass_kernel_spmd

dt = mybir.dt
AF = mybir.ActivationFunctionType
ALU = mybir.AluOpType
DRM = mybir.MatmulPerfMode.DoubleRow
F8 = dt.float8e4
F8NP = dt.np(F8)
F8MAX = float(_mld.finfo(F8NP).max)

L, B, S, D, H = 4, 2, 1024, 1024, 16
DH = D // H
F = 4 * D
ROPE_BASE = 10000.0
LN_EPS = 1e-5

N_CORES = 8
T = (B * S) // N_CORES            # 256 tokens per core
DC = D // 128                     # 8 feature chunks
FC_ = F // 128                    # 32 ffn chunks
G = 4                             # head groups (4 heads each)
HB = H * 65                       # V row: 16 heads x (64 feats + ones col)
GROUPS = [[0, 1, 2, 3], [4, 5, 6, 7]]
RANKS = 4
NEG = -60.0
CSUB = 3.0                        # constant subtracted pre-exp (softmax-inv)
QK8 = False                        # fp8 Q/K + DoubleRow scores
PHASE = int(os.environ.get("KPHASE", "9"))  # truncation knob for crash bisection
KATT = int(os.environ.get("KATT", "9"))    # attention sub-phase knob

_SHUF_MASK = [(i + 16) % 32 for i in range(32)]


def _f8(x):
    return np.clip(np.asarray(x, np.float32), -F8MAX, F8MAX).astype(F8NP)


def _qk_perm():
    """Per-head permutation: [16 even; 16 odd] rows per 32-row quadrant."""
    perm = np.zeros(D, dtype=np.int64)
    for h in range(H):
        for quad in range(2):
            for j in range(32):
                pair = quad * 16 + (j % 16)
                old_d = 2 * pair + (1 if j >= 16 else 0)
                perm[h * 64 + quad * 32 + j] = h * 64 + old_d
    return perm


def _rope_tables(core):
    """cosd/ssd [128, 2T] = v1 quadrant tables duplicated for the [q|k] pair."""
    j = core % RANKS
    pos = j * T + np.arange(T, dtype=np.float64)
    inv_freq = 1.0 / (ROPE_BASE ** (np.arange(0, DH, 2, dtype=np.float64) / DH))
    cos1 = np.zeros((128, T), dtype=np.float32)
    ss1 = np.zeros((128, T), dtype=np.float32)
    for p in range(128):
        qq, jj = p // 32, p % 32
        i = (qq % 2) * 16 + (jj % 16)
        ang = pos * inv_freq[i]
        cos1[p] = np.cos(ang)
        ss1[p] = (-np.sin(ang)) if jj < 16 else np.sin(ang)
    cosd = np.concatenate([cos1, cos1], axis=1)
    ssd = np.concatenate([ss1, ss1], axis=1)
    return cosd, ssd.astype(np.float16)


def _colmask(core):
    """[128, DC] f32 columns, constant per chunk: 0 if chunk before own window."""
    j = core % RANKS
    m = np.zeros((128, DC), dtype=np.float32)
    for kc in range(DC):
        vis = (kc + 1) * 128 <= j * T
        m[:, kc] = (0.0 if vis else NEG) - CSUB
    return m


def _trimask():
    """[128, 2T] f16: tri[p, k2*T + t] = (k2*128 + p) <= t; core-independent."""
    m = np.zeros((128, 2 * T), dtype=np.float32)
    for k2 in range(2):
        k = k2 * 128 + np.arange(128)
        m[:, k2 * T:(k2 + 1) * T] = (k[:, None] <= np.arange(T)[None, :])
    return m.astype(np.float16)


def build_program(has_bqk=False, has_bv=False, has_bfc=False, has_bpo=False):
    f16, f32 = dt.float16, dt.float32
    QKD = F8 if QK8 else f16
    KB = 128 * DC * T              # K elems per core
    VB = 128 * 2 * HB              # V elems per core (incl ones cols)

    nc = bacc.Bacc("TRN2", target_bir_lowering=False, debug=False,
                   num_devices=N_CORES)

    x0T = nc.dram_tensor("x0T", [D, T], f32, kind="ExternalInput")
    cosdT = nc.dram_tensor("cosdT", [128, 2 * T], f32, kind="ExternalInput")
    ssdT = nc.dram_tensor("ssdT", [128, 2 * T], f16, kind="ExternalInput")
    colmaskT = nc.dram_tensor("colmaskT", [128, DC], f32, kind="ExternalInput")
    triT = nc.dram_tensor("triT", [128, 2 * T], f16, kind="ExternalInput")
    wqk = nc.dram_tensor("wqk", [L, D, 2 * D], f16, kind="ExternalInput")
    wv = nc.dram_tensor("wv", [L, D, D], f16, kind="ExternalInput")
    wproj = nc.dram_tensor("wproj", [L, D, D], f16, kind="ExternalInput")
    wfc = nc.dram_tensor("wfc", [L, D, F], f16, kind="ExternalInput")
    wout = nc.dram_tensor("wout", [L, F, D], f16, kind="ExternalInput")
    bqk = nc.dram_tensor("bqk", [L, 128, 16], f32, kind="ExternalInput")
    bv = nc.dram_tensor("bv", [L, 1, D], f32, kind="ExternalInput")
    bproj = nc.dram_tensor("bproj", [L, 128, 8], f32, kind="ExternalInput")
    bfc = nc.dram_tensor("bfc", [L, 128, 32], f32, kind="ExternalInput")
    bout = nc.dram_tensor("bout", [L, 128, 8], f32, kind="ExternalInput")
    lnfg = nc.dram_tensor("lnfg", [128, 8], f32, kind="ExternalInput")
    lnfb = nc.dram_tensor("lnfb", [128, 8], f32, kind="ExternalInput")
    outT = nc.dram_tensor("outT", [D, T], f32, kind="ExternalOutput")

    from contextlib import ExitStack
    with ExitStack() as _es:
        tc = _es.enter_context(tile.TileContext(nc))
        pp = _es.enter_context(tc.tile_pool(name="persist", bufs=1))
        wqk_pool = _es.enter_context(tc.tile_pool(name="wqk", bufs=8))
        wv_pool = _es.enter_context(tc.tile_pool(name="wv", bufs=8))
        wsm_pool = _es.enter_context(tc.tile_pool(name="wsm", bufs=3))
        bias_pool = _es.enter_context(tc.tile_pool(name="bias", bufs=2))
        xh_pool = _es.enter_context(tc.tile_pool(name="xh", bufs=2))
        t16_pool = _es.enter_context(tc.tile_pool(name="ln16", bufs=8))
        rope_pool = _es.enter_context(tc.tile_pool(name="rope", bufs=2))
        t32_pool = _es.enter_context(tc.tile_pool(name="tmp32", bufs=2))
        probs_pool = _es.enter_context(tc.tile_pool(name="probs", bufs=6))
        stat_pool = _es.enter_context(tc.tile_pool(name="stat", bufs=2))
        bc_pool = _es.enter_context(tc.tile_pool(name="bcast", bufs=1))
        rb_pool = _es.enter_context(tc.tile_pool(name="rbp", bufs=2))
        ps_small = _es.enter_context(tc.tile_pool(name="ps_small", bufs=2, space="PSUM"))
        ps_bank = _es.enter_context(tc.tile_pool(name="ps_bank", bufs=2, space="PSUM"))
        ps_attn = _es.enter_context(tc.tile_pool(name="ps_attn", bufs=2, space="PSUM"))
        dram = _es.enter_context(tc.tile_pool(name="dram", bufs=1, space="DRAM"))
        if True:
            h_sb = pp.tile([128, DC * T], f32)
            cos_sb = pp.tile([128, 2 * T], f32)
            ss_sb = pp.tile([128, 2 * T], f16)
            colmask_sb = pp.tile([128, DC], f32)
            tri_sb = pp.tile([128, 2 * T], f16)
            QK_sb = pp.tile([128, DC, 2, T], QKD)     # [.., hp, (q|k), t]
            K_sb = pp.tile([128, DC, S], QKD)
            Vl_sb = pp.tile([128, 2, HB], f16)
            V_sb = pp.tile([128, DC, HB], f16)
            attn_sb = pp.tile([128, DC, T], f16)
            h1_sb = pp.tile([128, FC_, T], f16)
            outT_sb = pp.tile([128, DC * T], f32)
            ones_c = pp.tile([128, 1], f16)
            negc_c = pp.tile([128, 1], f32)
            zero_c = pp.tile([128, 1], f32)
            eps_c = pp.tile([1, 1], f32)
            lnfg_sb = pp.tile([128, 8], f32)
            lnfb_sb = pp.tile([128, 8], f32)

            kvlocK = dram.tile([KB], QKD)
            kvagK = dram.tile([RANKS * KB], QKD)
            kvlocV = dram.tile([VB], f16)
            kvagV = dram.tile([RANKS * VB], f16)

            nc.vector.memset(ones_c[:], 1.0)
            nc.vector.memset(negc_c[:], -CSUB)
            nc.vector.memset(zero_c[:], 0.0)
            nc.vector.memset(eps_c[:], LN_EPS)
            nc.vector.memset(Vl_sb[:], 1.0)
            nc.sync.dma_start(out=cos_sb[:], in_=cosdT[:])
            nc.sync.dma_start(out=ss_sb[:], in_=ssdT[:])
            nc.sync.dma_start(out=colmask_sb[:], in_=colmaskT[:])
            nc.sync.dma_start(out=tri_sb[:], in_=triT[:])
            nc.sync.dma_start(out=lnfg_sb[:], in_=lnfg[:])
            nc.sync.dma_start(out=lnfb_sb[:], in_=lnfb[:])
            nc.sync.dma_start(
                out=h_sb[:].rearrange("p (c t) -> p c t", t=T),
                in_=x0T.rearrange("(c p) t -> p c t", p=128),
            )
            if has_bv:
                bvb_sb = pp.tile([128, D], f32)

            def layer_norm(xhat):
                """h_sb (f32) -> xhat (f16 [128, DC, T]) = LN(h), no gamma/beta."""
                # p_s / p_sq share one PSUM bank: their accumulation groups
                # must run sequentially (start=True clears bits bank-wide)
                p_ss = ps_small.tile([1, 2 * T], f32, tag="ps_small")
                p_s = p_ss[0:1, 0:T]
                p_sq = p_ss[0:1, T:2 * T]
                h16s, sq16s = [], []
                for ci in range(DC):
                    hc = h_sb[:, ci * T:(ci + 1) * T]
                    h16 = t16_pool.tile([128, T], f16, tag="h16",
                                        name=f"h16_{ci}")
                    nc.vector.tensor_copy(h16[:], hc)
                    sq16 = t16_pool.tile([128, T], f16, tag="sq16",
                                         name=f"sq16_{ci}")
                    nc.vector.tensor_tensor(out=sq16[:], in0=h16[:], in1=h16[:],
                                            op=ALU.mult)
                    h16s.append(h16)
                    sq16s.append(sq16)
                for ci in range(DC):
                    nc.tensor.matmul(p_s, ones_c[:], h16s[ci][:],
                                     start=(ci == 0), stop=(ci == DC - 1))
                for ci in range(DC):
                    nc.tensor.matmul(p_sq, ones_c[:], sq16s[ci][:],
                                     start=(ci == 0), stop=(ci == DC - 1))
                m = stat_pool.tile([1, T], f32, tag="st_m")
                msq = stat_pool.tile([1, T], f32, tag="st_msq")
                var = stat_pool.tile([1, T], f32, tag="st_var")
                rstd = stat_pool.tile([1, T], f32, tag="st_rstd")
                mr = stat_pool.tile([1, T], f32, tag="st_mr")
                nc.vector.tensor_scalar_mul(m[:], p_s, 1.0 / D)
                nc.vector.tensor_scalar_mul(msq[:], p_sq, 1.0 / D)
                nc.vector.tensor_tensor(out=var[:], in0=m[:], in1=m[:], op=ALU.mult)
                nc.vector.tensor_sub(var[:], msq[:], var[:])
                # rstd = exp(-0.5 * ln(var + eps)) — stays on the ln/exp table
                nc.scalar.activation(var[:], var[:], AF.Ln, bias=eps_c[:])
                nc.vector.tensor_scalar_mul(var[:], var[:], -0.5)
                nc.scalar.activation(rstd[:], var[:], AF.Exp)
                nc.vector.tensor_tensor(out=mr[:], in0=m[:], in1=rstd[:], op=ALU.mult)
                rstd_b = bc_pool.tile([128, T], f32, tag="rstd_b")
                mr_b = bc_pool.tile([128, T], f32, tag="mr_b")
                nc.gpsimd.partition_broadcast(rstd_b[:], rstd[:])
                nc.gpsimd.partition_broadcast(mr_b[:], mr[:])
                for ci in range(DC):
                    hc = h_sb[:, ci * T:(ci + 1) * T]
                    u = t32_pool.tile([128, T], f32, tag="ln_u")
                    nc.vector.tensor_tensor(out=u[:], in0=hc, in1=rstd_b[:],
                                            op=ALU.mult)
                    nc.vector.tensor_tensor(out=xhat[:, ci, :],
                                            in0=u[:], in1=mr_b[:], op=ALU.subtract)

            for l in range(L):
                # ---- per-layer bias tiles ----
                bqk_t = bias_pool.tile([128, 16], f32, tag="bqk")
                bproj_t = bias_pool.tile([128, 8], f32, tag="bproj")
                bfc_t = bias_pool.tile([128, 32], f32, tag="bfc")
                bout_t = bias_pool.tile([128, 8], f32, tag="bout")
                nc.sync.dma_start(out=bqk_t[:], in_=bqk[l])
                nc.sync.dma_start(out=bproj_t[:], in_=bproj[l])
                nc.sync.dma_start(out=bfc_t[:], in_=bfc[l])
                nc.sync.dma_start(out=bout_t[:], in_=bout[l])
                if has_bv:
                    bv_t = bias_pool.tile([1, D], f32, tag="bv")
                    nc.sync.dma_start(out=bv_t[:], in_=bv[l])
                    nc.gpsimd.partition_broadcast(bvb_sb[:], bv_t[:])

                # ---- LN1 ----
                xhat = xh_pool.tile([128, DC, T], f16, tag="xhat")
                layer_norm(xhat)

                # ---- QKV projections (q,k with RoPE; v token-major) ----
                wqk_t = [wqk_pool.tile([128, 2 * D], f16, tag="wqk",
                                       name=f"wqk_t{k}") for k in range(DC)]
                wv_t = [wv_pool.tile([128, D], f16, tag="wv", name=f"wv_t{k}")
                        for k in range(DC)]
                for k in range(DC):
                    nc.sync.dma_start(out=wqk_t[k][:],
                                      in_=wqk[l, k * 128:(k + 1) * 128, :])
                    nc.sync.dma_start(out=wv_t[k][:],
                                      in_=wv[l, k * 128:(k + 1) * 128, :])
                for cs in range(DC):               # head-pair chunk of q AND k
                    pq = ps_bank.tile([128, 4 * T], f32, tag="ps_bank", name="pq")
                    # q in bank0 [0:T], k in bank1 [2T:3T] — accumulation
                    # groups interleaved in ONE bank corrupt each other
                    # (start=True clears accumulate bits bank-wide)
                    for k in range(DC):
                        nc.tensor.matmul(
                            pq[:, 0:T], wqk_t[k][:, cs * 128:(cs + 1) * 128],
                            xhat[:, k, :],
                            start=(k == 0), stop=(k == DC - 1))
                        nc.tensor.matmul(
                            pq[:, 2 * T:3 * T],
                            wqk_t[k][:, D + cs * 128:D + (cs + 1) * 128],
                            xhat[:, k, :],
                            start=(k == 0), stop=(k == DC - 1))
                    dslc = QK_sb[:, cs, :, :]          # [128, 2, T]
                    pqv = pq[:].rearrange("p (b h t) -> p b h t",
                                          b=2, h=2, t=T)[:, :, 0, :]
                    cosw = cos_sb[:].rearrange("p (b t) -> p b t", t=T)
                    ssw = ss_sb[:].rearrange("p (b t) -> p b t", t=T)
                    qtmp = rope_pool.tile([128, 2, T], f16, tag="rope_q")
                    ctmp = rope_pool.tile([128, 2, T], f16, tag="rope_c")
                    stmp = rope_pool.tile([128, 2 * T], f16, tag="rope_s")
                    dtmp = rope_pool.tile([128, 2, T], f16, tag="rope_d")
                    if has_bqk:
                        nc.vector.tensor_scalar_add(
                            qtmp[:, 0, :], pq[:, 0:T], bqk_t[:, cs:cs + 1])
                        nc.vector.tensor_scalar_add(
                            qtmp[:, 1, :], pq[:, 2 * T:3 * T],
                            bqk_t[:, 8 + cs:9 + cs])
                        nc.vector.tensor_tensor(out=ctmp[:], in0=qtmp[:],
                                                in1=cosw, op=ALU.mult)
                    else:
                        nc.vector.tensor_copy(qtmp[:], pqv)
                        nc.vector.tensor_tensor(out=ctmp[:], in0=pqv,
                                                in1=cosw, op=ALU.mult)
                    qtf = qtmp[:].rearrange("p b t -> p (b t)")
                    stmpv = stmp[:].rearrange("p (b t) -> p b t", t=T)
                    nc.vector.stream_shuffle(stmp[:], qtf, _SHUF_MASK)
                    nc.vector.tensor_tensor(out=dtmp[:], in0=stmpv, in1=ssw,
                                            op=ALU.mult)
                    nc.vector.tensor_tensor(out=dslc, in0=ctmp[:], in1=dtmp[:],
                                            op=ALU.add)
                if PHASE < 1:
                    continue
                # K AllGather first — scores overlap the V exchange
                nc.sync.dma_start(
                    out=kvlocK.rearrange("(c p t) -> p c t", p=128, t=T),
                    in_=QK_sb[:, :, 1, :],
                )
                nc.gpsimd.collective_compute(
                    "AllGather", ALU.bypass,
                    ins=[kvlocK.opt()], outs=[kvagK.opt()],
                    replica_groups=GROUPS,
                )
                if PHASE < 2:
                    continue
                # v: token-major [T, D] via lhsT = xhat slices
                for tci in range(2):
                    for fh in range(2):
                        p_v = ps_attn.tile([128, 512], f32, tag="ps_attn")
                        for k in range(DC):
                            nc.tensor.matmul(
                                p_v[:],
                                xhat[:, k, tci * 128:(tci + 1) * 128],
                                wv_t[k][:, fh * 512:(fh + 1) * 512],
                                start=(k == 0), stop=(k == DC - 1),
                            )
                        vdst = Vl_sb[:, tci, :].rearrange(
                            "p (h f) -> p h f", f=65)[:, fh * 8:(fh + 1) * 8, 0:64]
                        if has_bv:
                            nc.vector.tensor_tensor(
                                out=vdst, in0=p_v[:].rearrange("p (h f) -> p h f", f=64),
                                in1=bvb_sb[:, fh * 512:(fh + 1) * 512].rearrange(
                                    "p (h f) -> p h f", f=64),
                                op=ALU.add)
                        else:
                            nc.vector.tensor_copy(
                                vdst, p_v[:].rearrange("p (h f) -> p h f", f=64))
                if PHASE < 3:
                    continue
                nc.sync.dma_start(
                    out=kvlocV.rearrange("(c p f) -> p c f", p=128, f=HB),
                    in_=Vl_sb[:],
                )
                nc.gpsimd.collective_compute(
                    "AllGather", ALU.bypass,
                    ins=[kvlocV.opt()], outs=[kvagV.opt()],
                    replica_groups=GROUPS,
                )
                for rr in range(RANKS):
                    nc.sync.dma_start(
                        out=K_sb[:, :, rr * T:(rr + 1) * T],
                        in_=kvagK[rr * KB:(rr + 1) * KB].rearrange(
                            "(c p t) -> p c t", p=128, t=T),
                    )
                    nc.sync.dma_start(
                        out=V_sb[:, rr * 2:(rr + 1) * 2, :],
                        in_=kvagV[rr * VB:(rr + 1) * VB].rearrange(
                            "(c p f) -> p c f", p=128, f=HB),
                    )

                if PHASE < 4:
                    continue
                # ---- attention ----
                Qv = QK_sb[:, :, 0, :]
                Klv = QK_sb[:, :, 1, :]
                for hp in range(DC):
                    if True:
                        # pa: both heads share one bank but their AV
                        # accumulation groups run sequentially, so start's
                        # bank-wide accumulate-bit clear cannot corrupt
                        pa = ps_attn.tile([128, 2 * T], f32, tag="ps_attn",
                                          name="pa")
                        p_at = [pa[0:65, i * T:(i + 1) * T] for i in range(2)]
                        probs_l = []
                        for kcp in range(5):
                            own = kcp == 4
                            probs = probs_pool.tile([128, 2, 2 * T], f16,
                                                    tag="probs",
                                                    name=f"probs{kcp}")
                            probs_l.append(probs)
                            # one matmul output region per PSUM bank:
                            # tile k2, region hh*2T
                            p_s = [ps_bank.tile([128, 4 * T], f32, tag="ps_bank",
                                                name=f"p_s{i}") for i in range(2)]
                            for k2 in range(2):
                                kc = 2 * kcp + k2
                                for hh in range(2):
                                    bp = 64 * hh
                                    if own:
                                        kslc = Klv[bp:bp + 64, hp,
                                                   k2 * 128:(k2 + 1) * 128]
                                    else:
                                        kslc = K_sb[bp:bp + 64, hp,
                                                    kc * 128:(kc + 1) * 128]
                                    qslc = Qv[bp:bp + 64, hp, :]
                                    po = p_s[k2][:, hh * 2 * T:hh * 2 * T + T]
                                    nc.tensor.matmul(po, kslc, qslc,
                                                     start=True, stop=True)
                            bias = (negc_c[:, 0:1] if own
                                    else colmask_sb[:, 2 * kcp:2 * kcp + 1])
                            if KATT < 2:
                                continue
                            for k2 in range(2):
                                nc.scalar.activation(
                                    probs[:, k2, :].rearrange(
                                        "p (b t) -> p b t", t=T),
                                    p_s[k2][:].rearrange(
                                        "p (b h t) -> p b h t",
                                        b=2, h=2, t=T)[:, :, 0, :],
                                    AF.Exp, bias=bias)
                            if own:
                                for k2 in range(2):
                                    for hh in range(2):
                                        pslc = probs[:, k2, hh * T:(hh + 1) * T]
                                        nc.vector.tensor_tensor(
                                            out=pslc, in0=pslc,
                                            in1=tri_sb[:, k2 * T:(k2 + 1) * T],
                                            op=ALU.mult)
                        for hh in range(KATT >= 4 and 2 or 0):
                            hgl = 2 * hp + hh
                            for kcp in range(5):
                                own = kcp == 4
                                Vsrc = (Vl_sb[:, :, hgl * 65:hgl * 65 + 65] if own
                                        else V_sb[:, 2 * kcp:2 * kcp + 2,
                                                  hgl * 65:hgl * 65 + 65])
                                rhs = probs_l[kcp][:, :, hh * T:(hh + 1) * T]
                                for k2 in range(2):
                                    nc.tensor.matmul(
                                        p_at[hh][0:65, :], Vsrc[:, k2, :],
                                        rhs[:, k2, :],
                                        start=(kcp == 0 and k2 == 0),
                                        stop=(own and k2 == 1))
                        if KATT < 5:
                            continue
                        recip = stat_pool.tile([1, 2 * T], f32, tag="recip")
                        for hh in range(2):
                            nc.vector.reciprocal(
                                recip[:, hh * T:(hh + 1) * T], p_at[hh][64:65, :])
                        rb = rb_pool.tile([64, 2 * T], f32, tag="rb")
                        nc.gpsimd.partition_broadcast(rb[:], recip[:])
                        for hh in range(2):
                            nc.vector.tensor_tensor(
                                out=attn_sb[hh * 64:(hh + 1) * 64, hp, :],
                                in0=p_at[hh][0:64, :],
                                in1=rb[:, hh * T:(hh + 1) * T],
                                op=ALU.mult,
                            )

                if PHASE < 5:
                    continue
                # ---- attention out-proj + residual ----
                # 4 concurrent dj accumulation groups live in 4 distinct
                # banks: tile (dj//2), region (dj%2)*2T
                def qreg(tiles, dj):
                    return tiles[dj // 2][:, (dj % 2) * 2 * T:
                                          (dj % 2) * 2 * T + T]

                def qview(tl):
                    return tl[:].rearrange("p (b h t) -> p b h t",
                                           b=2, h=2, t=T)[:, :, 0, :]

                for half in range(2):
                    p_pr = [ps_bank.tile([128, 4 * T], f32, tag="ps_bank",
                                         name=f"p_pr{i}") for i in range(2)]
                    for k in range(DC):
                        wproj_t = wsm_pool.tile([128, 512], f16, tag="wproj")
                        nc.sync.dma_start(
                            out=wproj_t[:],
                            in_=wproj[l, k * 128:(k + 1) * 128,
                                      half * 512:(half + 1) * 512])
                        for dj in range(4):
                            nc.tensor.matmul(
                                qreg(p_pr, dj),
                                wproj_t[:, dj * 128:(dj + 1) * 128],
                                attn_sb[:, k, :],
                                start=(k == 0), stop=(k == DC - 1),
                            )
                    if has_bpo:
                        for dj in range(4):
                            dci = half * 4 + dj
                            nc.vector.scalar_tensor_tensor(
                                out=h_sb[:, dci * T:(dci + 1) * T],
                                in0=qreg(p_pr, dj),
                                scalar=bproj_t[:, dci:dci + 1],
                                in1=h_sb[:, dci * T:(dci + 1) * T],
                                op0=ALU.add, op1=ALU.add,
                            )
                    else:
                        for tt in range(2):
                            c0 = (half * 4 + 2 * tt) * T
                            hs = h_sb[:, c0:c0 + 2 * T].rearrange(
                                "p (b t) -> p b t", t=T)
                            nc.vector.tensor_tensor(
                                out=hs, in0=qview(p_pr[tt]), in1=hs, op=ALU.add)

                if PHASE < 6:
                    continue
                # ---- LN2 ----
                xhat2 = xh_pool.tile([128, DC, T], f16, tag="xhat")
                layer_norm(xhat2)

                # ---- FFN: fc + gelu -> h1, then out-proj + residual ----
                for gg in range(F // 512):         # 8 groups of 4 output chunks
                    p_fc = [ps_bank.tile([128, 4 * T], f32, tag="ps_bank",
                                         name=f"p_fc{i}") for i in range(2)]
                    for k in range(DC):
                        wfc_t = wsm_pool.tile([128, 512], f16, tag="wfc")
                        nc.sync.dma_start(
                            out=wfc_t[:],
                            in_=wfc[l, k * 128:(k + 1) * 128,
                                    gg * 512:(gg + 1) * 512])
                        for fj in range(4):
                            nc.tensor.matmul(
                                qreg(p_fc, fj),
                                wfc_t[:, fj * 128:(fj + 1) * 128],
                                xhat2[:, k, :],
                                start=(k == 0), stop=(k == DC - 1),
                            )
                    if has_bfc:
                        for fj in range(4):
                            fci = gg * 4 + fj
                            nc.scalar.activation(
                                h1_sb[:, fci, :],
                                qreg(p_fc, fj),
                                AF.Gelu_apprx_tanh,
                                bias=bfc_t[:, fci:fci + 1],
                            )
                    else:
                        for tt in range(2):
                            nc.scalar.activation(
                                h1_sb[:, gg * 4 + 2 * tt:gg * 4 + 2 * tt + 2, :],
                                qview(p_fc[tt]),
                                AF.Gelu_apprx_tanh,
                                bias=zero_c[:, 0:1],
                            )
                if PHASE < 7:
                    continue
                for half in range(2):
                    p_o = [ps_bank.tile([128, 4 * T], f32, tag="ps_bank",
                                        name=f"p_o{i}") for i in range(2)]
                    for k in range(FC_):           # 32 contraction chunks
                        wout_t = wsm_pool.tile([128, 512], f16, tag="wout")
                        nc.sync.dma_start(
                            out=wout_t[:],
                            in_=wout[l, k * 128:(k + 1) * 128,
                                     half * 512:(half + 1) * 512])
                        for dj in range(4):
                            nc.tensor.matmul(
                                qreg(p_o, dj),
                                wout_t[:, dj * 128:(dj + 1) * 128],
                                h1_sb[:, k, :],
                                start=(k == 0), stop=(k == FC_ - 1),
                            )
                    if has_bpo:
                        for dj in range(4):
                            dci = half * 4 + dj
                            nc.vector.scalar_tensor_tensor(
                                out=h_sb[:, dci * T:(dci + 1) * T],
                                in0=qreg(p_o, dj),
                                scalar=bout_t[:, dci:dci + 1],
                                in1=h_sb[:, dci * T:(dci + 1) * T],
                                op0=ALU.add, op1=ALU.add,
                            )
                    else:
                        for tt in range(2):
                            c0 = (half * 4 + 2 * tt) * T
                            hs = h_sb[:, c0:c0 + 2 * T].rearrange(
                                "p (b t) -> p b t", t=T)
                            nc.vector.tensor_tensor(
                                out=hs, in0=qview(p_o[tt]), in1=hs, op=ALU.add)

            # ---- final LN with gamma/beta, fp32 apply ----
            p_ss = ps_small.tile([1, 2 * T], f32, tag="ps_small")
            p_s = p_ss[0:1, 0:T]
            p_sq = p_ss[0:1, T:2 * T]
            h16s, sq16s = [], []
            for ci in range(DC):
                hc = h_sb[:, ci * T:(ci + 1) * T]
                h16 = t16_pool.tile([128, T], f16, tag="h16", name=f"h16f{ci}")
                nc.vector.tensor_copy(h16[:], hc)
                sq16 = t16_pool.tile([128, T], f16, tag="sq16", name=f"sq16f{ci}")
                nc.vector.tensor_tensor(out=sq16[:], in0=h16[:], in1=h16[:],
                                        op=ALU.mult)
                h16s.append(h16)
                sq16s.append(sq16)
            for ci in range(DC):
                nc.tensor.matmul(p_s, ones_c[:], h16s[ci][:],
                                 start=(ci == 0), stop=(ci == DC - 1))
            for ci in range(DC):
                nc.tensor.matmul(p_sq, ones_c[:], sq16s[ci][:],
                                 start=(ci == 0), stop=(ci == DC - 1))
            m = stat_pool.tile([1, T], f32, tag="st_m")
            msq = stat_pool.tile([1, T], f32, tag="st_msq")
            var = stat_pool.tile([1, T], f32, tag="st_var")
            rstd = stat_pool.tile([1, T], f32, tag="st_rstd")
            mr = stat_pool.tile([1, T], f32, tag="st_mr")
            nc.vector.tensor_scalar_mul(m[:], p_s, 1.0 / D)
            nc.vector.tensor_scalar_mul(msq[:], p_sq, 1.0 / D)
            nc.vector.tensor_tensor(out=var[:], in0=m[:], in1=m[:], op=ALU.mult)
            nc.vector.tensor_sub(var[:], msq[:], var[:])
            nc.scalar.activation(var[:], var[:], AF.Ln, bias=eps_c[:])
            nc.vector.tensor_scalar_mul(var[:], var[:], -0.5)
            nc.scalar.activation(rstd[:], var[:], AF.Exp)
            nc.vector.tensor_tensor(out=mr[:], in0=m[:], in1=rstd[:], op=ALU.mult)
            rstd_b = bc_pool.tile([128, T], f32, tag="rstd_b")
            mr_b = bc_pool.tile([128, T], f32, tag="mr_b")
            nc.gpsimd.partition_broadcast(rstd_b[:], rstd[:])
            nc.gpsimd.partition_broadcast(mr_b[:], mr[:])
            for ci in range(DC):
                hc = h_sb[:, ci * T:(ci + 1) * T]
                u = t32_pool.tile([128, T], f32, tag="ln_u")
                z = t32_pool.tile([128, T], f32, tag="ln_z")
                nc.vector.tensor_tensor(out=u[:], in0=hc, in1=rstd_b[:], op=ALU.mult)
                nc.vector.tensor_tensor(out=z[:], in0=u[:], in1=mr_b[:],
                                        op=ALU.subtract)
                nc.vector.tensor_scalar(
                    out=outT_sb[:, ci * T:(ci + 1) * T], in0=z[:],
                    scalar1=lnfg_sb[:, ci:ci + 1], scalar2=lnfb_sb[:, ci:ci + 1],
                    op0=ALU.mult, op1=ALU.add,
                )
            nc.sync.dma_start(
                out=outT.rearrange("(c p) t -> p c t", p=128),
                in_=outT_sb[:].rearrange("p (c t) -> p c t", t=T),
            )

    nc.compile()
    return nc


_CACHED = {}


def _prep_inputs(inputs_embeds, w_qkv, b_qkv, w_proj, b_proj, w_fc, b_fc,
                 w_out, b_out, ln1_g, ln1_b, ln2_g, ln2_b, lnf_g, lnf_b):
    """Fold LN gamma/beta into weights; permute+scale q/k; cast fp16."""
    perm = _qk_perm()
    rs = np.sqrt(0.125)
    f16 = np.float16

    wqk_l, wv_l, bqk_l, bv_l = [], [], [], []
    wfc_l, bfc_l = [], []
    for l in range(L):
        b_eff = b_qkv[l] + ln1_b[l] @ w_qkv[l]          # [3D]
        w_eff = ln1_g[l][:, None] * w_qkv[l]            # [D, 3D]
        wq = w_eff[:, perm] * rs
        wk = w_eff[:, D + perm] * rs
        bq = b_eff[perm] * rs
        bk = b_eff[D + perm] * rs
        wqk_l.append(np.concatenate([wq, wk], axis=1).astype(f16))
        wv_l.append(w_eff[:, 2 * D:].astype(f16))
        bqk_l.append(np.concatenate([bq, bk]).reshape(16, 128).T.astype(np.float32))
        bv_l.append(b_eff[2 * D:].reshape(1, D).astype(np.float32))
        bfc_eff = b_fc[l] + ln2_b[l] @ w_fc[l]
        wfc_l.append((ln2_g[l][:, None] * w_fc[l]).astype(f16))
        bfc_l.append(bfc_eff.reshape(32, 128).T.astype(np.float32))
    shared = {
        "wqk": np.stack(wqk_l),
        "wv": np.stack(wv_l),
        "wproj": np.asarray(w_proj).astype(f16),
        "wfc": np.stack(wfc_l),
        "wout": np.asarray(w_out).astype(f16),
        "bqk": np.stack(bqk_l),
        "bv": np.stack(bv_l),
        "bproj": b_proj.reshape(L, 8, 128).transpose(0, 2, 1).astype(np.float32),
        "bfc": np.stack(bfc_l),
        "bout": b_out.reshape(L, 8, 128).transpose(0, 2, 1).astype(np.float32),
        "lnfg": lnf_g.reshape(8, 128).T.astype(np.float32),
        "lnfb": lnf_b.reshape(8, 128).T.astype(np.float32),
        "triT": _trimask(),
    }
    flags = dict(
        has_bqk=bool(np.any(shared["bqk"])),
        has_bv=bool(np.any(shared["bv"])),
        has_bfc=bool(np.any(shared["bfc"])),
        has_bpo=bool(np.any(shared["bproj"])) or bool(np.any(shared["bout"])),
    )
    x_flat = np.asarray(inputs_embeds, dtype=np.float32).reshape(B * S, D)
    in_maps = []
    for c in range(N_CORES):
        cosd, ssd = _rope_tables(c)
        m = dict(shared)
        m["x0T"] = np.ascontiguousarray(x_flat[c * T:(c + 1) * T].T)
        m["cosdT"] = cosd
        m["ssdT"] = ssd
        m["colmaskT"] = _colmask(c)
        in_maps.append(m)
    return in_maps, flags


def kernel(**inputs):
    inputs = {k: np.asarray(v) for k, v in inputs.items()}
    in_maps, flags = _prep_inputs(
        inputs["inputs_embeds"], inputs["w_qkv"], inputs["b_qkv"],
        inputs["w_proj"], inputs["b_proj"], inputs["w_fc"], inputs["b_fc"],
        inputs["w_out"], inputs["b_out"], inputs["ln1_g"], inputs["ln1_b"],
        inputs["ln2_g"], inputs["ln2_b"], inputs["lnf_g"], inputs["lnf_b"],
    )
    key = ("nc",) + tuple(sorted(flags.items()))
    if key not in _CACHED:
        _CACHED[key] = build_program(**flags)
    _CACHED["nc"] = _CACHED[key]
    res = run_bass_kernel_spmd(_CACHED[key], in_maps, list(range(N_CORES)))
    out = np.empty((B * S, D), dtype=np.float32)
    for c in range(N_CORES):
        out[c * T:(c + 1) * T] = res.results[c]["outT"].T
    return out.reshape(B, S, D)


if __name__ == "__main__":
    print("building program...")
    build_program()
    print("built OK")


# revision 31
# speedup vs baseline: 1.0629x; 1.0629x over previous
"""Trainium2 Bass kernel for a 4-layer dense transformer (B=2, S=1024, D=1024, H=16).

Sharding: context-parallel over tokens across 8 cores (256 tokens/core;
cores 0-3 = batch 0, cores 4-7 = batch 1). Per layer, K and V are exchanged
within each 4-core batch group via two AllGathers (K first, so score compute
overlaps the V exchange); everything else is local.

Layouts: feature-major residual h^T [D, T]; GEMMs consume weights in native
[in, out] layout as the stationary operand (fp16, fp32 PSUM accumulate).
Attention: Q/K are fp8e4 in a per-head [32 partition rows x 2 free slots]
layout so scores run as one DoubleRow matmul per (head, key chunk). Causal
masking: key chunks never straddle a core's query window, so remote chunks
take a per-chunk-pair constant bias folded into the softmax exp (own chunks
masked out there), and a core-independent triangular mask handles the
diagonal via a separate own-chunk pass on the pre-AllGather K/V. The softmax
denominator rides along as a 65th ones-column of V. PSUM is managed as
2-bank [128, 1024] tiles so exp / gelu / residual adds run as single wide
ops. Residual, stats fp32.
"""

import sys
import os

for _p in ("/opt/trn_rl_repo", "/root/.axon_site/_ro/trn_rl_repo"):
    if os.path.isdir(_p) and _p not in sys.path:
        sys.path.insert(0, _p)

import numpy as np
import ml_dtypes as _mld
import concourse.bass as bass
import concourse.bacc as bacc
import concourse.mybir as mybir
import concourse.tile as tile
from concourse.bass_utils import run_bass_kernel_spmd

dt = mybir.dt
AF = mybir.ActivationFunctionType
ALU = mybir.AluOpType
DRM = mybir.MatmulPerfMode.DoubleRow
F8 = dt.float8e4
F8NP = dt.np(F8)
F8MAX = float(_mld.finfo(F8NP).max)

L, B, S, D, H = 4, 2, 1024, 1024, 16
DH = D // H
F = 4 * D
ROPE_BASE = 10000.0
LN_EPS = 1e-5

N_CORES = 8
T = (B * S) // N_CORES            # 256 tokens per core
DC = D // 128                     # 8 feature chunks
FC_ = F // 128                    # 32 ffn chunks
G = 4                             # head groups (4 heads each)
HB = H * 65                       # V row: 16 heads x (64 feats + ones col)
GROUPS = [[0, 1, 2, 3], [4, 5, 6, 7]]
RANKS = 4
NEG = -60.0
CSUB = 3.0                        # constant subtracted pre-exp (softmax-inv)
QK8 = True                        # fp8 Q/K + DoubleRow scores
PHASE = int(os.environ.get("KPHASE", "9"))  # truncation knob for crash bisection
KATT = int(os.environ.get("KATT", "9"))    # attention sub-phase knob

_SHUF_MASK = [(i + 16) % 32 for i in range(32)]


def _f8(x):
    return np.clip(np.asarray(x, np.float32), -F8MAX, F8MAX).astype(F8NP)


def _qk_perm():
    """Per-head permutation: [16 even; 16 odd] rows per 32-row quadrant."""
    perm = np.zeros(D, dtype=np.int64)
    for h in range(H):
        for quad in range(2):
            for j in range(32):
                pair = quad * 16 + (j % 16)
                old_d = 2 * pair + (1 if j >= 16 else 0)
                perm[h * 64 + quad * 32 + j] = h * 64 + old_d
    return perm


def _rope_tables(core):
    """cosd/ssd [128, 2T] = v1 quadrant tables duplicated for the [q|k] pair."""
    j = core % RANKS
    pos = j * T + np.arange(T, dtype=np.float64)
    inv_freq = 1.0 / (ROPE_BASE ** (np.arange(0, DH, 2, dtype=np.float64) / DH))
    cos1 = np.zeros((128, T), dtype=np.float32)
    ss1 = np.zeros((128, T), dtype=np.float32)
    for p in range(128):
        qq, jj = p // 32, p % 32
        i = (qq % 2) * 16 + (jj % 16)
        ang = pos * inv_freq[i]
        cos1[p] = np.cos(ang)
        ss1[p] = (-np.sin(ang)) if jj < 16 else np.sin(ang)
    cosd = np.concatenate([cos1, cos1], axis=1)
    ssd = np.concatenate([ss1, ss1], axis=1)
    return cosd, ssd.astype(np.float16)


def _colmask(core):
    """[128, DC] f32 columns, constant per chunk: 0 if chunk before own window."""
    j = core % RANKS
    m = np.zeros((128, DC), dtype=np.float32)
    for kc in range(DC):
        vis = (kc + 1) * 128 <= j * T
        m[:, kc] = (0.0 if vis else NEG) - CSUB
    return m


def _trimask():
    """[128, 2T] f16: tri[p, k2*T + t] = (k2*128 + p) <= t; core-independent."""
    m = np.zeros((128, 2 * T), dtype=np.float32)
    for k2 in range(2):
        k = k2 * 128 + np.arange(128)
        m[:, k2 * T:(k2 + 1) * T] = (k[:, None] <= np.arange(T)[None, :])
    return m.astype(np.float16)


def build_program(has_bqk=False, has_bv=False, has_bfc=False, has_bpo=False):
    f16, f32 = dt.float16, dt.float32
    QKD = F8 if QK8 else f16
    KB = 128 * DC * T              # K elems per core
    VB = 128 * 2 * HB              # V elems per core (incl ones cols)

    nc = bacc.Bacc("TRN2", target_bir_lowering=False, debug=False,
                   num_devices=N_CORES)

    x0T = nc.dram_tensor("x0T", [D, T], f32, kind="ExternalInput")
    cosdT = nc.dram_tensor("cosdT", [128, 2 * T], f32, kind="ExternalInput")
    ssdT = nc.dram_tensor("ssdT", [128, 2 * T], f16, kind="ExternalInput")
    colmaskT = nc.dram_tensor("colmaskT", [128, DC], f32, kind="ExternalInput")
    triT = nc.dram_tensor("triT", [128, 2 * T], f16, kind="ExternalInput")
    wqk = nc.dram_tensor("wqk", [L, D, 2 * D], f16, kind="ExternalInput")
    wv = nc.dram_tensor("wv", [L, D, D], f16, kind="ExternalInput")
    wproj = nc.dram_tensor("wproj", [L, D, D], f16, kind="ExternalInput")
    wfc = nc.dram_tensor("wfc", [L, D, F], f16, kind="ExternalInput")
    wout = nc.dram_tensor("wout", [L, F, D], f16, kind="ExternalInput")
    bqk = nc.dram_tensor("bqk", [L, 128, 16], f32, kind="ExternalInput")
    bv = nc.dram_tensor("bv", [L, 1, D], f32, kind="ExternalInput")
    bproj = nc.dram_tensor("bproj", [L, 128, 8], f32, kind="ExternalInput")
    bfc = nc.dram_tensor("bfc", [L, 128, 32], f32, kind="ExternalInput")
    bout = nc.dram_tensor("bout", [L, 128, 8], f32, kind="ExternalInput")
    lnfg = nc.dram_tensor("lnfg", [128, 8], f32, kind="ExternalInput")
    lnfb = nc.dram_tensor("lnfb", [128, 8], f32, kind="ExternalInput")
    outT = nc.dram_tensor("outT", [D, T], f32, kind="ExternalOutput")

    from contextlib import ExitStack
    with ExitStack() as _es:
        tc = _es.enter_context(tile.TileContext(nc))
        pp = _es.enter_context(tc.tile_pool(name="persist", bufs=1))
        wqk_pool = _es.enter_context(tc.tile_pool(name="wqk", bufs=8))
        wv_pool = _es.enter_context(tc.tile_pool(name="wv", bufs=8))
        wsm_pool = _es.enter_context(tc.tile_pool(name="wsm", bufs=3))
        bias_pool = _es.enter_context(tc.tile_pool(name="bias", bufs=2))
        xh_pool = _es.enter_context(tc.tile_pool(name="xh", bufs=2))
        t16_pool = _es.enter_context(tc.tile_pool(name="ln16", bufs=8))
        rope_pool = _es.enter_context(tc.tile_pool(name="rope", bufs=2))
        t32_pool = _es.enter_context(tc.tile_pool(name="tmp32", bufs=2))
        probs_pool = _es.enter_context(tc.tile_pool(name="probs", bufs=6))
        stat_pool = _es.enter_context(tc.tile_pool(name="stat", bufs=2))
        bc_pool = _es.enter_context(tc.tile_pool(name="bcast", bufs=1))
        rb_pool = _es.enter_context(tc.tile_pool(name="rbp", bufs=2))
        ps_small = _es.enter_context(tc.tile_pool(name="ps_small", bufs=2, space="PSUM"))
        ps_bank = _es.enter_context(tc.tile_pool(name="ps_bank", bufs=2, space="PSUM"))
        ps_attn = _es.enter_context(tc.tile_pool(name="ps_attn", bufs=2, space="PSUM"))
        dram = _es.enter_context(tc.tile_pool(name="dram", bufs=1, space="DRAM"))
        if True:
            h_sb = pp.tile([128, DC * T], f32)
            cos_sb = pp.tile([128, 2 * T], f32)
            ss_sb = pp.tile([128, 2 * T], f16)
            colmask_sb = pp.tile([128, DC], f32)
            tri_sb = pp.tile([128, 2 * T], f16)
            QK_sb = pp.tile([128, DC, 2, T], QKD)     # [.., hp, (q|k), t]
            K_sb = pp.tile([128, DC, S], QKD)
            Vl_sb = pp.tile([128, 2, HB], f16)
            V_sb = pp.tile([128, DC, HB], f16)
            attn_sb = pp.tile([128, DC, T], f16)
            h1_sb = pp.tile([128, FC_, T], f16)
            outT_sb = pp.tile([128, DC * T], f32)
            ones_c = pp.tile([128, 1], f16)
            negc_c = pp.tile([128, 1], f32)
            zero_c = pp.tile([128, 1], f32)
            eps_c = pp.tile([1, 1], f32)
            lnfg_sb = pp.tile([128, 8], f32)
            lnfb_sb = pp.tile([128, 8], f32)

            kvlocK = dram.tile([KB], QKD)
            kvagK = dram.tile([RANKS * KB], QKD)
            kvlocV = dram.tile([VB], f16)
            kvagV = dram.tile([RANKS * VB], f16)

            nc.vector.memset(ones_c[:], 1.0)
            nc.vector.memset(negc_c[:], -CSUB)
            nc.vector.memset(zero_c[:], 0.0)
            nc.vector.memset(eps_c[:], LN_EPS)
            nc.vector.memset(Vl_sb[:], 1.0)
            nc.sync.dma_start(out=cos_sb[:], in_=cosdT[:])
            nc.sync.dma_start(out=ss_sb[:], in_=ssdT[:])
            nc.sync.dma_start(out=colmask_sb[:], in_=colmaskT[:])
            nc.sync.dma_start(out=tri_sb[:], in_=triT[:])
            nc.sync.dma_start(out=lnfg_sb[:], in_=lnfg[:])
            nc.sync.dma_start(out=lnfb_sb[:], in_=lnfb[:])
            nc.sync.dma_start(
                out=h_sb[:].rearrange("p (c t) -> p c t", t=T),
                in_=x0T.rearrange("(c p) t -> p c t", p=128),
            )
            if has_bv:
                bvb_sb = pp.tile([128, D], f32)

            def layer_norm(xhat):
                """h_sb (f32) -> xhat (f16 [128, DC, T]) = LN(h), no gamma/beta."""
                # p_s / p_sq share one PSUM bank: their accumulation groups
                # must run sequentially (start=True clears bits bank-wide)
                p_ss = ps_small.tile([1, 2 * T], f32, tag="ps_small")
                p_s = p_ss[0:1, 0:T]
                p_sq = p_ss[0:1, T:2 * T]
                h16s, sq16s = [], []
                for ci in range(DC):
                    hc = h_sb[:, ci * T:(ci + 1) * T]
                    h16 = t16_pool.tile([128, T], f16, tag="h16",
                                        name=f"h16_{ci}")
                    nc.vector.tensor_copy(h16[:], hc)
                    sq16 = t16_pool.tile([128, T], f16, tag="sq16",
                                         name=f"sq16_{ci}")
                    nc.vector.tensor_tensor(out=sq16[:], in0=h16[:], in1=h16[:],
                                            op=ALU.mult)
                    h16s.append(h16)
                    sq16s.append(sq16)
                for ci in range(DC):
                    nc.tensor.matmul(p_s, ones_c[:], h16s[ci][:],
                                     start=(ci == 0), stop=(ci == DC - 1))
                for ci in range(DC):
                    nc.tensor.matmul(p_sq, ones_c[:], sq16s[ci][:],
                                     start=(ci == 0), stop=(ci == DC - 1))
                m = stat_pool.tile([1, T], f32, tag="st_m")
                msq = stat_pool.tile([1, T], f32, tag="st_msq")
                var = stat_pool.tile([1, T], f32, tag="st_var")
                rstd = stat_pool.tile([1, T], f32, tag="st_rstd")
                mr = stat_pool.tile([1, T], f32, tag="st_mr")
                nc.vector.tensor_scalar_mul(m[:], p_s, 1.0 / D)
                nc.vector.tensor_scalar_mul(msq[:], p_sq, 1.0 / D)
                nc.vector.tensor_tensor(out=var[:], in0=m[:], in1=m[:], op=ALU.mult)
                nc.vector.tensor_sub(var[:], msq[:], var[:])
                # rstd = exp(-0.5 * ln(var + eps)) — stays on the ln/exp table
                nc.scalar.activation(var[:], var[:], AF.Ln, bias=eps_c[:])
                nc.vector.tensor_scalar_mul(var[:], var[:], -0.5)
                nc.scalar.activation(rstd[:], var[:], AF.Exp)
                nc.vector.tensor_tensor(out=mr[:], in0=m[:], in1=rstd[:], op=ALU.mult)
                rstd_b = bc_pool.tile([128, T], f32, tag="rstd_b")
                mr_b = bc_pool.tile([128, T], f32, tag="mr_b")
                nc.gpsimd.partition_broadcast(rstd_b[:], rstd[:])
                nc.gpsimd.partition_broadcast(mr_b[:], mr[:])
                for ci in range(DC):
                    hc = h_sb[:, ci * T:(ci + 1) * T]
                    u = t32_pool.tile([128, T], f32, tag="ln_u")
                    nc.vector.tensor_tensor(out=u[:], in0=hc, in1=rstd_b[:],
                                            op=ALU.mult)
                    nc.vector.tensor_tensor(out=xhat[:, ci, :],
                                            in0=u[:], in1=mr_b[:], op=ALU.subtract)

            for l in range(L):
                # ---- per-layer bias tiles ----
                bqk_t = bias_pool.tile([128, 16], f32, tag="bqk")
                bproj_t = bias_pool.tile([128, 8], f32, tag="bproj")
                bfc_t = bias_pool.tile([128, 32], f32, tag="bfc")
                bout_t = bias_pool.tile([128, 8], f32, tag="bout")
                nc.sync.dma_start(out=bqk_t[:], in_=bqk[l])
                nc.sync.dma_start(out=bproj_t[:], in_=bproj[l])
                nc.sync.dma_start(out=bfc_t[:], in_=bfc[l])
                nc.sync.dma_start(out=bout_t[:], in_=bout[l])
                if has_bv:
                    bv_t = bias_pool.tile([1, D], f32, tag="bv")
                    nc.sync.dma_start(out=bv_t[:], in_=bv[l])
                    nc.gpsimd.partition_broadcast(bvb_sb[:], bv_t[:])

                # ---- LN1 ----
                xhat = xh_pool.tile([128, DC, T], f16, tag="xhat")
                layer_norm(xhat)

                # ---- QKV projections (q,k with RoPE; v token-major) ----
                wqk_t = [wqk_pool.tile([128, 2 * D], f16, tag="wqk",
                                       name=f"wqk_t{k}") for k in range(DC)]
                wv_t = [wv_pool.tile([128, D], f16, tag="wv", name=f"wv_t{k}")
                        for k in range(DC)]
                for k in range(DC):
                    nc.sync.dma_start(out=wqk_t[k][:],
                                      in_=wqk[l, k * 128:(k + 1) * 128, :])
                    nc.sync.dma_start(out=wv_t[k][:],
                                      in_=wv[l, k * 128:(k + 1) * 128, :])
                for cs in range(DC):               # head-pair chunk of q AND k
                    pq = ps_bank.tile([128, 4 * T], f32, tag="ps_bank", name="pq")
                    # q in bank0 [0:T], k in bank1 [2T:3T] — accumulation
                    # groups interleaved in ONE bank corrupt each other
                    # (start=True clears accumulate bits bank-wide)
                    for k in range(DC):
                        nc.tensor.matmul(
                            pq[:, 0:T], wqk_t[k][:, cs * 128:(cs + 1) * 128],
                            xhat[:, k, :],
                            start=(k == 0), stop=(k == DC - 1))
                        nc.tensor.matmul(
                            pq[:, 2 * T:3 * T],
                            wqk_t[k][:, D + cs * 128:D + (cs + 1) * 128],
                            xhat[:, k, :],
                            start=(k == 0), stop=(k == DC - 1))
                    dslc = QK_sb[:, cs, :, :]          # [128, 2, T]
                    pqv = pq[:].rearrange("p (b h t) -> p b h t",
                                          b=2, h=2, t=T)[:, :, 0, :]
                    cosw = cos_sb[:].rearrange("p (b t) -> p b t", t=T)
                    ssw = ss_sb[:].rearrange("p (b t) -> p b t", t=T)
                    qtmp = rope_pool.tile([128, 2, T], f16, tag="rope_q")
                    ctmp = rope_pool.tile([128, 2, T], f16, tag="rope_c")
                    stmp = rope_pool.tile([128, 2 * T], f16, tag="rope_s")
                    dtmp = rope_pool.tile([128, 2, T], f16, tag="rope_d")
                    if has_bqk:
                        nc.vector.tensor_scalar_add(
                            qtmp[:, 0, :], pq[:, 0:T], bqk_t[:, cs:cs + 1])
                        nc.vector.tensor_scalar_add(
                            qtmp[:, 1, :], pq[:, 2 * T:3 * T],
                            bqk_t[:, 8 + cs:9 + cs])
                        nc.vector.tensor_tensor(out=ctmp[:], in0=qtmp[:],
                                                in1=cosw, op=ALU.mult)
                    else:
                        nc.vector.tensor_copy(qtmp[:], pqv)
                        nc.vector.tensor_tensor(out=ctmp[:], in0=pqv,
                                                in1=cosw, op=ALU.mult)
                    qtf = qtmp[:].rearrange("p b t -> p (b t)")
                    stmpv = stmp[:].rearrange("p (b t) -> p b t", t=T)
                    nc.vector.stream_shuffle(stmp[:], qtf, _SHUF_MASK)
                    nc.vector.tensor_tensor(out=dtmp[:], in0=stmpv, in1=ssw,
                                            op=ALU.mult)
                    nc.vector.tensor_tensor(out=dslc, in0=ctmp[:], in1=dtmp[:],
                                            op=ALU.add)
                if PHASE < 1:
                    continue
                # K AllGather first — scores overlap the V exchange
                nc.sync.dma_start(
                    out=kvlocK.rearrange("(c p t) -> p c t", p=128, t=T),
                    in_=QK_sb[:, :, 1, :],
                )
                nc.gpsimd.collective_compute(
                    "AllGather", ALU.bypass,
                    ins=[kvlocK.opt()], outs=[kvagK.opt()],
                    replica_groups=GROUPS,
                )
                if PHASE < 2:
                    continue
                # v: token-major [T, D] via lhsT = xhat slices
                for tci in range(2):
                    for fh in range(2):
                        p_v = ps_attn.tile([128, 512], f32, tag="ps_attn")
                        for k in range(DC):
                            nc.tensor.matmul(
                                p_v[:],
                                xhat[:, k, tci * 128:(tci + 1) * 128],
                                wv_t[k][:, fh * 512:(fh + 1) * 512],
                                start=(k == 0), stop=(k == DC - 1),
                            )
                        vdst = Vl_sb[:, tci, :].rearrange(
                            "p (h f) -> p h f", f=65)[:, fh * 8:(fh + 1) * 8, 0:64]
                        if has_bv:
                            nc.vector.tensor_tensor(
                                out=vdst, in0=p_v[:].rearrange("p (h f) -> p h f", f=64),
                                in1=bvb_sb[:, fh * 512:(fh + 1) * 512].rearrange(
                                    "p (h f) -> p h f", f=64),
                                op=ALU.add)
                        else:
                            nc.vector.tensor_copy(
                                vdst, p_v[:].rearrange("p (h f) -> p h f", f=64))
                if PHASE < 3:
                    continue
                nc.sync.dma_start(
                    out=kvlocV.rearrange("(c p f) -> p c f", p=128, f=HB),
                    in_=Vl_sb[:],
                )
                nc.gpsimd.collective_compute(
                    "AllGather", ALU.bypass,
                    ins=[kvlocV.opt()], outs=[kvagV.opt()],
                    replica_groups=GROUPS,
                )
                for rr in range(RANKS):
                    nc.sync.dma_start(
                        out=K_sb[:, :, rr * T:(rr + 1) * T],
                        in_=kvagK[rr * KB:(rr + 1) * KB].rearrange(
                            "(c p t) -> p c t", p=128, t=T),
                    )
                    nc.sync.dma_start(
                        out=V_sb[:, rr * 2:(rr + 1) * 2, :],
                        in_=kvagV[rr * VB:(rr + 1) * VB].rearrange(
                            "(c p f) -> p c f", p=128, f=HB),
                    )

                if PHASE < 4:
                    continue
                # ---- attention ----
                Qv = QK_sb[:, :, 0, :]
                Klv = QK_sb[:, :, 1, :]
                for hp in range(DC):
                    if True:
                        # pa: both heads share one bank but their AV
                        # accumulation groups run sequentially, so start's
                        # bank-wide accumulate-bit clear cannot corrupt
                        pa = ps_attn.tile([128, 2 * T], f32, tag="ps_attn",
                                          name="pa")
                        p_at = [pa[0:65, i * T:(i + 1) * T] for i in range(2)]
                        probs_l = []
                        for kcp in range(5):
                            own = kcp == 4
                            probs = probs_pool.tile([128, 2, 2 * T], f16,
                                                    tag="probs",
                                                    name=f"probs{kcp}")
                            probs_l.append(probs)
                            # one matmul output region per PSUM bank:
                            # tile k2, region hh*2T
                            p_s = [ps_bank.tile([128, 4 * T], f32, tag="ps_bank",
                                                name=f"p_s{i}") for i in range(2)]
                            for k2 in range(2):
                                kc = 2 * kcp + k2
                                for hh in range(2):
                                    bp = 64 * hh
                                    if own:
                                        kslc = Klv[bp:bp + 64, hp,
                                                   k2 * 128:(k2 + 1) * 128]
                                    else:
                                        kslc = K_sb[bp:bp + 64, hp,
                                                    kc * 128:(kc + 1) * 128]
                                    qslc = Qv[bp:bp + 64, hp, :]
                                    po = p_s[k2][:, hh * 2 * T:hh * 2 * T + T]
                                    nc.tensor.matmul(po, kslc, qslc,
                                                     start=True, stop=True)
                            bias = (negc_c[:, 0:1] if own
                                    else colmask_sb[:, 2 * kcp:2 * kcp + 1])
                            if KATT < 2:
                                continue
                            for k2 in range(2):
                                nc.scalar.activation(
                                    probs[:, k2, :].rearrange(
                                        "p (b t) -> p b t", t=T),
                                    p_s[k2][:].rearrange(
                                        "p (b h t) -> p b h t",
                                        b=2, h=2, t=T)[:, :, 0, :],
                                    AF.Exp, bias=bias)
                            if own:
                                for k2 in range(2):
                                    for hh in range(2):
                                        pslc = probs[:, k2, hh * T:(hh + 1) * T]
                                        nc.vector.tensor_tensor(
                                            out=pslc, in0=pslc,
                                            in1=tri_sb[:, k2 * T:(k2 + 1) * T],
                                            op=ALU.mult)
                        for hh in range(KATT >= 4 and 2 or 0):
                            hgl = 2 * hp + hh
                            for kcp in range(5):
                                own = kcp == 4
                                Vsrc = (Vl_sb[:, :, hgl * 65:hgl * 65 + 65] if own
                                        else V_sb[:, 2 * kcp:2 * kcp + 2,
                                                  hgl * 65:hgl * 65 + 65])
                                rhs = probs_l[kcp][:, :, hh * T:(hh + 1) * T]
                                for k2 in range(2):
                                    nc.tensor.matmul(
                                        p_at[hh][0:65, :], Vsrc[:, k2, :],
                                        rhs[:, k2, :],
                                        start=(kcp == 0 and k2 == 0),
                                        stop=(own and k2 == 1))
                        if KATT < 5:
                            continue
                        recip = stat_pool.tile([1, 2 * T], f32, tag="recip")
                        for hh in range(2):
                            nc.vector.reciprocal(
                                recip[:, hh * T:(hh + 1) * T], p_at[hh][64:65, :])
                        rb = rb_pool.tile([64, 2 * T], f32, tag="rb")
                        nc.gpsimd.partition_broadcast(rb[:], recip[:])
                        for hh in range(2):
                            nc.vector.tensor_tensor(
                                out=attn_sb[hh * 64:(hh + 1) * 64, hp, :],
                                in0=p_at[hh][0:64, :],
                                in1=rb[:, hh * T:(hh + 1) * T],
                                op=ALU.mult,
                            )

                if PHASE < 5:
                    continue
                # ---- attention out-proj + residual ----
                # 4 concurrent dj accumulation groups live in 4 distinct
                # banks: tile (dj//2), region (dj%2)*2T
                def qreg(tiles, dj):
                    return tiles[dj // 2][:, (dj % 2) * 2 * T:
                                          (dj % 2) * 2 * T + T]

                def qview(tl):
                    return tl[:].rearrange("p (b h t) -> p b h t",
                                           b=2, h=2, t=T)[:, :, 0, :]

                for half in range(2):
                    p_pr = [ps_bank.tile([128, 4 * T], f32, tag="ps_bank",
                                         name=f"p_pr{i}") for i in range(2)]
                    for k in range(DC):
                        wproj_t = wsm_pool.tile([128, 512], f16, tag="wproj")
                        nc.sync.dma_start(
                            out=wproj_t[:],
                            in_=wproj[l, k * 128:(k + 1) * 128,
                                      half * 512:(half + 1) * 512])
                        for dj in range(4):
                            nc.tensor.matmul(
                                qreg(p_pr, dj),
                                wproj_t[:, dj * 128:(dj + 1) * 128],
                                attn_sb[:, k, :],
                                start=(k == 0), stop=(k == DC - 1),
                            )
                    if has_bpo:
                        for dj in range(4):
                            dci = half * 4 + dj
                            nc.vector.scalar_tensor_tensor(
                                out=h_sb[:, dci * T:(dci + 1) * T],
                                in0=qreg(p_pr, dj),
                                scalar=bproj_t[:, dci:dci + 1],
                                in1=h_sb[:, dci * T:(dci + 1) * T],
                                op0=ALU.add, op1=ALU.add,
                            )
                    else:
                        for tt in range(2):
                            c0 = (half * 4 + 2 * tt) * T
                            hs = h_sb[:, c0:c0 + 2 * T].rearrange(
                                "p (b t) -> p b t", t=T)
                            nc.vector.tensor_tensor(
                                out=hs, in0=qview(p_pr[tt]), in1=hs, op=ALU.add)

                if PHASE < 6:
                    continue
                # ---- LN2 ----
                xhat2 = xh_pool.tile([128, DC, T], f16, tag="xhat")
                layer_norm(xhat2)

                # ---- FFN: fc + gelu -> h1, then out-proj + residual ----
                for gg in range(F // 512):         # 8 groups of 4 output chunks
                    p_fc = [ps_bank.tile([128, 4 * T], f32, tag="ps_bank",
                                         name=f"p_fc{i}") for i in range(2)]
                    for k in range(DC):
                        wfc_t = wsm_pool.tile([128, 512], f16, tag="wfc")
                        nc.sync.dma_start(
                            out=wfc_t[:],
                            in_=wfc[l, k * 128:(k + 1) * 128,
                                    gg * 512:(gg + 1) * 512])
                        for fj in range(4):
                            nc.tensor.matmul(
                                qreg(p_fc, fj),
                                wfc_t[:, fj * 128:(fj + 1) * 128],
                                xhat2[:, k, :],
                                start=(k == 0), stop=(k == DC - 1),
                            )
                    if has_bfc:
                        for fj in range(4):
                            fci = gg * 4 + fj
                            nc.scalar.activation(
                                h1_sb[:, fci, :],
                                qreg(p_fc, fj),
                                AF.Gelu_apprx_tanh,
                                bias=bfc_t[:, fci:fci + 1],
                            )
                    else:
                        for tt in range(2):
                            nc.scalar.activation(
                                h1_sb[:, gg * 4 + 2 * tt:gg * 4 + 2 * tt + 2, :],
                                qview(p_fc[tt]),
                                AF.Gelu_apprx_tanh,
                                bias=zero_c[:, 0:1],
                            )
                if PHASE < 7:
                    continue
                for half in range(2):
                    p_o = [ps_bank.tile([128, 4 * T], f32, tag="ps_bank",
                                        name=f"p_o{i}") for i in range(2)]
                    for k in range(FC_):           # 32 contraction chunks
                        wout_t = wsm_pool.tile([128, 512], f16, tag="wout")
                        nc.sync.dma_start(
                            out=wout_t[:],
                            in_=wout[l, k * 128:(k + 1) * 128,
                                     half * 512:(half + 1) * 512])
                        for dj in range(4):
                            nc.tensor.matmul(
                                qreg(p_o, dj),
                                wout_t[:, dj * 128:(dj + 1) * 128],
                                h1_sb[:, k, :],
                                start=(k == 0), stop=(k == FC_ - 1),
                            )
                    if has_bpo:
                        for dj in range(4):
                            dci = half * 4 + dj
                            nc.vector.scalar_tensor_tensor(
                                out=h_sb[:, dci * T:(dci + 1) * T],
                                in0=qreg(p_o, dj),
                                scalar=bout_t[:, dci:dci + 1],
                                in1=h_sb[:, dci * T:(dci + 1) * T],
                                op0=ALU.add, op1=ALU.add,
                            )
                    else:
                        for tt in range(2):
                            c0 = (half * 4 + 2 * tt) * T
                            hs = h_sb[:, c0:c0 + 2 * T].rearrange(
                                "p (b t) -> p b t", t=T)
                            nc.vector.tensor_tensor(
                                out=hs, in0=qview(p_o[tt]), in1=hs, op=ALU.add)

            # ---- final LN with gamma/beta, fp32 apply ----
            p_ss = ps_small.tile([1, 2 * T], f32, tag="ps_small")
            p_s = p_ss[0:1, 0:T]
            p_sq = p_ss[0:1, T:2 * T]
            h16s, sq16s = [], []
            for ci in range(DC):
                hc = h_sb[:, ci * T:(ci + 1) * T]
                h16 = t16_pool.tile([128, T], f16, tag="h16", name=f"h16f{ci}")
                nc.vector.tensor_copy(h16[:], hc)
                sq16 = t16_pool.tile([128, T], f16, tag="sq16", name=f"sq16f{ci}")
                nc.vector.tensor_tensor(out=sq16[:], in0=h16[:], in1=h16[:],
                                        op=ALU.mult)
                h16s.append(h16)
                sq16s.append(sq16)
            for ci in range(DC):
                nc.tensor.matmul(p_s, ones_c[:], h16s[ci][:],
                                 start=(ci == 0), stop=(ci == DC - 1))
            for ci in range(DC):
                nc.tensor.matmul(p_sq, ones_c[:], sq16s[ci][:],
                                 start=(ci == 0), stop=(ci == DC - 1))
            m = stat_pool.tile([1, T], f32, tag="st_m")
            msq = stat_pool.tile([1, T], f32, tag="st_msq")
            var = stat_pool.tile([1, T], f32, tag="st_var")
            rstd = stat_pool.tile([1, T], f32, tag="st_rstd")
            mr = stat_pool.tile([1, T], f32, tag="st_mr")
            nc.vector.tensor_scalar_mul(m[:], p_s, 1.0 / D)
            nc.vector.tensor_scalar_mul(msq[:], p_sq, 1.0 / D)
            nc.vector.tensor_tensor(out=var[:], in0=m[:], in1=m[:], op=ALU.mult)
            nc.vector.tensor_sub(var[:], msq[:], var[:])
            nc.scalar.activation(var[:], var[:], AF.Ln, bias=eps_c[:])
            nc.vector.tensor_scalar_mul(var[:], var[:], -0.5)
            nc.scalar.activation(rstd[:], var[:], AF.Exp)
            nc.vector.tensor_tensor(out=mr[:], in0=m[:], in1=rstd[:], op=ALU.mult)
            rstd_b = bc_pool.tile([128, T], f32, tag="rstd_b")
            mr_b = bc_pool.tile([128, T], f32, tag="mr_b")
            nc.gpsimd.partition_broadcast(rstd_b[:], rstd[:])
            nc.gpsimd.partition_broadcast(mr_b[:], mr[:])
            for ci in range(DC):
                hc = h_sb[:, ci * T:(ci + 1) * T]
                u = t32_pool.tile([128, T], f32, tag="ln_u")
                z = t32_pool.tile([128, T], f32, tag="ln_z")
                nc.vector.tensor_tensor(out=u[:], in0=hc, in1=rstd_b[:], op=ALU.mult)
                nc.vector.tensor_tensor(out=z[:], in0=u[:], in1=mr_b[:],
                                        op=ALU.subtract)
                nc.vector.tensor_scalar(
                    out=outT_sb[:, ci * T:(ci + 1) * T], in0=z[:],
                    scalar1=lnfg_sb[:, ci:ci + 1], scalar2=lnfb_sb[:, ci:ci + 1],
                    op0=ALU.mult, op1=ALU.add,
                )
            nc.sync.dma_start(
                out=outT.rearrange("(c p) t -> p c t", p=128),
                in_=outT_sb[:].rearrange("p (c t) -> p c t", t=T),
            )

    nc.compile()
    return nc


_CACHED = {}


def _prep_inputs(inputs_embeds, w_qkv, b_qkv, w_proj, b_proj, w_fc, b_fc,
                 w_out, b_out, ln1_g, ln1_b, ln2_g, ln2_b, lnf_g, lnf_b):
    """Fold LN gamma/beta into weights; permute+scale q/k; cast fp16."""
    perm = _qk_perm()
    rs = np.sqrt(0.125)
    f16 = np.float16

    wqk_l, wv_l, bqk_l, bv_l = [], [], [], []
    wfc_l, bfc_l = [], []
    for l in range(L):
        b_eff = b_qkv[l] + ln1_b[l] @ w_qkv[l]          # [3D]
        w_eff = ln1_g[l][:, None] * w_qkv[l]            # [D, 3D]
        wq = w_eff[:, perm] * rs
        wk = w_eff[:, D + perm] * rs
        bq = b_eff[perm] * rs
        bk = b_eff[D + perm] * rs
        wqk_l.append(np.concatenate([wq, wk], axis=1).astype(f16))
        wv_l.append(w_eff[:, 2 * D:].astype(f16))
        bqk_l.append(np.concatenate([bq, bk]).reshape(16, 128).T.astype(np.float32))
        bv_l.append(b_eff[2 * D:].reshape(1, D).astype(np.float32))
        bfc_eff = b_fc[l] + ln2_b[l] @ w_fc[l]
        wfc_l.append((ln2_g[l][:, None] * w_fc[l]).astype(f16))
        bfc_l.append(bfc_eff.reshape(32, 128).T.astype(np.float32))
    shared = {
        "wqk": np.stack(wqk_l),
        "wv": np.stack(wv_l),
        "wproj": np.asarray(w_proj).astype(f16),
        "wfc": np.stack(wfc_l),
        "wout": np.asarray(w_out).astype(f16),
        "bqk": np.stack(bqk_l),
        "bv": np.stack(bv_l),
        "bproj": b_proj.reshape(L, 8, 128).transpose(0, 2, 1).astype(np.float32),
        "bfc": np.stack(bfc_l),
        "bout": b_out.reshape(L, 8, 128).transpose(0, 2, 1).astype(np.float32),
        "lnfg": lnf_g.reshape(8, 128).T.astype(np.float32),
        "lnfb": lnf_b.reshape(8, 128).T.astype(np.float32),
        "triT": _trimask(),
    }
    flags = dict(
        has_bqk=bool(np.any(shared["bqk"])),
        has_bv=bool(np.any(shared["bv"])),
        has_bfc=bool(np.any(shared["bfc"])),
        has_bpo=bool(np.any(shared["bproj"])) or bool(np.any(shared["bout"])),
    )
    x_flat = np.asarray(inputs_embeds, dtype=np.float32).reshape(B * S, D)
    in_maps = []
    for c in range(N_CORES):
        cosd, ssd = _rope_tables(c)
        m = dict(shared)
        m["x0T"] = np.ascontiguousarray(x_flat[c * T:(c + 1) * T].T)
        m["cosdT"] = cosd
        m["ssdT"] = ssd
        m["colmaskT"] = _colmask(c)
        in_maps.append(m)
    return in_maps, flags


def kernel(**inputs):
    inputs = {k: np.asarray(v) for k, v in inputs.items()}
    in_maps, flags = _prep_inputs(
        inputs["inputs_embeds"], inputs["w_qkv"], inputs["b_qkv"],
        inputs["w_proj"], inputs["b_proj"], inputs["w_fc"], inputs["b_fc"],
        inputs["w_out"], inputs["b_out"], inputs["ln1_g"], inputs["ln1_b"],
        inputs["ln2_g"], inputs["ln2_b"], inputs["lnf_g"], inputs["lnf_b"],
    )
    key = ("nc",) + tuple(sorted(flags.items()))
    if key not in _CACHED:
        _CACHED[key] = build_program(**flags)
    _CACHED["nc"] = _CACHED[key]
    res = run_bass_kernel_spmd(_CACHED[key], in_maps, list(range(N_CORES)))
    out = np.empty((B * S, D), dtype=np.float32)
    for c in range(N_CORES):
        out[c * T:(c + 1) * T] = res.results[c]["outT"].T
    return out.reshape(B, S, D)


if __name__ == "__main__":
    print("building program...")
    build_program()
    print("built OK")
